# revision 48
# baseline (speedup 1.0000x reference)
"""Dot-product attention (B=32, S=2048, D=64, per-batch key masking) on 8 trn2 cores.

Strategy: valid_lens makes keys >= valid_len contribute exactly zero
(exp(-1e6) == 0 in f32), so fully-masked 128-key chunks are skipped entirely.
Work is scheduled as K fixed-size "slots" per core (SPMD: every core runs the
same program); each slot instance processes one piece = (batch, chunk-range)
of up to slot-size chunks against that batch's full 2048 queries, producing a
partial [65, 2048] = (numerator^T ; denominator) that the host sums per batch
and divides. Batches are split across cores/slots to balance the load
(~Sum(ceil(vl/128))/8 chunks per core instead of 4*16).

The host pre-transposes and pre-casts operands (Q^T|K^T in bf16, V augmented
with a ones column) so the device program is just: DMA in, then per chunk
S^T = K_c @ Q^T on PE -> exp on ScalarE (mask as bias) -> oT += V'_c^T @ exp
on PE, then drain oT partials and DMA out.
"""

import sys

import numpy as np

_TRN_REPO = "/opt/trn_rl_repo"
if _TRN_REPO not in sys.path:
    sys.path.insert(0, _TRN_REPO)

B, S, D = 32, 2048, 64
N_CORES = 8
NT = S // 128  # 16 query row-tiles
NEG = -1000000.0

_CACHE = {}
_FORCE_CAND = None  # test hook: index into plan_candidates


# ---------------------------------------------------------------- scheduling


def _feasible(sizes, chunks, n_cores=8):
    avail = []
    for k, s in enumerate(sizes):
        for _ in range(n_cores):
            avail.append([s, k])
    order = sorted(range(len(chunks)), key=lambda b: -chunks[b])
    pieces = []
    for b in order:
        r = chunks[b]
        lo = 0
        while r > 0:
            if not avail:
                return None
            geq = [i for i, (sz, _) in enumerate(avail) if sz >= r]
            if geq:
                i = min(geq, key=lambda i: avail[i][0])
                sz, k = avail.pop(i)
                pieces.append((b, lo, r, k))
                lo += r
                r = 0
            else:
                i = max(range(len(avail)), key=lambda i: avail[i][0])
                sz, k = avail.pop(i)
                if sz == 0:
                    return None
                pieces.append((b, lo, sz, k))
                lo += sz
                r -= sz
    return pieces


def _partitions(total, parts, max_v):
    if parts == 1:
        if 1 <= total <= max_v:
            yield (total,)
        return
    lo = -(-total // parts)
    for v in range(min(max_v, total - (parts - 1)), lo - 1, -1):
        for rest in _partitions(total - v, parts - 1, v):
            yield (v,) + rest


def plan_candidates(chunks, n_cores=8, max_extra=6, max_chunk=16):
    total_lb = -(-sum(chunks) // n_cores)
    out = []
    for total in range(total_lb, total_lb + max_extra + 1):
        for K in (4, 5, 6, 7):
            if K * n_cores < len(chunks):
                continue
            best_for_k = None
            for sizes in _partitions(total, K, max_chunk):
                pieces = _feasible(sizes, chunks, n_cores)
                if pieces is not None:
                    key = (sizes[-1], sizes)
                    if best_for_k is None or key > best_for_k[0]:
                        best_for_k = (key, sizes, pieces)
            if best_for_k:
                out.append((total, K, best_for_k[1], best_for_k[2]))
    return out


def _plan(chunks):
    """Returns (sizes, assign): assign[core][slot] = (batch, lo, ln) or None."""
    cands = plan_candidates(chunks)
    if _FORCE_CAND is None:
        # chunk work dominates; each extra slot costs ~1 chunk of overhead
        # (empirically calibrated against TimelineSim)
        pick = min(cands, key=lambda c: c[0] + 1.0 * c[1])
    else:
        pick = cands[_FORCE_CAND]
    total, K, sizes, pieces = pick
    assign = [[None] * K for _ in range(N_CORES)]
    nxt = [0] * K
    for b, lo, ln, k in pieces:
        assign[nxt[k]][k] = (b, lo, ln)
        nxt[k] += 1

    # Emission order matters (TimelineSim-calibrated): non-1 slots ascending
    # (largest last -> clean tail), size-1 slots interleaved between the
    # leading small slots so their PSUM-drain bursts are absorbed mid-stream.
    ones = [i for i in range(K) if sizes[i] == 1]
    others = sorted((i for i in range(K) if sizes[i] > 1), key=lambda i: sizes[i])
    order = []
    oi = 0
    for j, i in enumerate(others):
        order.append(i)
        if j >= 0 and oi < len(ones) and j < len(others) - 1:
            order.append(ones[oi])
            oi += 1
    order.extend(ones[oi:])
    if not others:
        order = list(range(K))
    sizes2 = tuple(sizes[i] for i in order)
    assign2 = [[assign[core][i] for i in order] for core in range(N_CORES)]
    return sizes2, assign2


# ------------------------------------------------------------------- program


def _build_nc(sizes):
    import concourse.bacc as bacc
    import concourse.mybir as mybir
    import concourse.tile as tile

    f32 = mybir.dt.float32
    bf16 = mybir.dt.bfloat16
    Exp = mybir.ActivationFunctionType.Exp

    nc = bacc.Bacc()
    K = len(sizes)

    # qk{m}: [65, s*128 + S] = K^T | Q^T augmented with a mask row (bf16):
    # row 64 of K^T holds 0 / -1e6 per key, row 64 of Q^T is 1.0, so the
    # scores matmul (contraction 65) applies the key mask directly.
    qk_d = [
        nc.dram_tensor(f"qk{m}", [65, sizes[m] * 128 + S], bf16, kind="ExternalInput")
        for m in range(K)
    ]
    # vt{m}: [128, s*65] = V chunk-major with ones column per chunk
    vtb_d = [
        nc.dram_tensor(
            f"vtb{m}", [128, sizes[m] * (D + 1)], bf16, kind="ExternalInput"
        )
        for m in range(K)
    ]
    # fast-path input for the very first chunk-half: K^T chunk 0 | Q^T half 0
    fast0_d = nc.dram_tensor("fast0", [65, 128 + 1024], bf16, kind="ExternalInput")
    out_d = [
        nc.dram_tensor(f"out{m}", [65, S], bf16, kind="ExternalOutput") for m in range(K)
    ]

    with tile.TileContext(nc) as tc:
        with (
            tc.tile_pool(name="warm", bufs=1) as warmp,
            tc.tile_pool(name="qkp", bufs=3) as qkp,
            tc.tile_pool(name="vtp", bufs=3) as vtp,
            tc.tile_pool(name="biasp", bufs=4) as biasp,
            tc.tile_pool(name="expp", bufs=6) as expp,
            tc.tile_pool(name="fin", bufs=2) as finp,
            tc.tile_pool(name="psc", bufs=2, space="PSUM") as psc,
            tc.tile_pool(name="pso", bufs=4, space="PSUM") as pso,
        ):
            # trigger the exp act-table load off the critical path
            warm = warmp.tile([1, 2], f32, name="warm", tag="warm")
            nc.vector.memset(warm[:, 0:1], 0.0)
            nc.scalar.activation(warm[:, 1:2], warm[:, 0:1], Exp)
            # PE p-state warmup: dummy matmuls on a zeroed tile while the
            # first real input DMA is in flight
            wmm = warmp.tile([64, 640], bf16, name="wmm", tag="wmm")
            nc.gpsimd.memset(wmm[:], 0.0)
            wps = psc.tile([128, 1024], f32, name="sc", tag="sc")
            for jj in range(4):
                nc.tensor.matmul(
                    wps[:, 512 * (jj % 2) : 512 * (jj % 2 + 1)],
                    wmm[:, 0:128],
                    wmm[:, 128:640],
                    start=True,
                    stop=True,
                )

            # fast-path tiles for the first chunk-half
            fast0 = warmp.tile([65, 128 + 1024], bf16, name="fast0", tag="fast0")
            nc.sync.dma_start(fast0[:], fast0_d[:])

            # per-slot state, filled lazily
            slot_t = [None] * K
            halves = [
                (m, c, h) for m, s in enumerate(sizes) for c in range(s) for h in (0, 1)
            ]
            N = len(halves)
            sc_t = [None] * N
            ex_t = [None] * N

            def ensure_loaded(m):
                if slot_t[m] is not None:
                    return slot_t[m]
                s = sizes[m]
                qk = qkp.tile([65, s * 128 + S], bf16, name="qk", tag="qk")
                if m == 0:
                    # chunk-0 K^T / first q-half come via the fast-path tile;
                    # one DMA for the rest (re-covers unused q-half bytes)
                    nc.sync.dma_start(qk[:, 128:], qk_d[m][:, 128:])
                else:
                    nc.sync.dma_start(qk[:], qk_d[m][:])
                kt = qk[:, 0 : s * 128]
                qt = qk[:, s * 128 :]
                vtb = vtp.tile([128, s * (D + 1)], bf16, name="vtb", tag="vtb")
                nc.sync.dma_start(vtb[:], vtb_d[m][:])
                vt3 = vtb.rearrange("p (c w) -> p c w", w=D + 1)
                oT = [
                    pso.tile([65, 512], f32, name=f"oT{j}", tag="oT") for j in range(4)
                ]
                slot_t[m] = {
                    "qt": qt,
                    "kt": kt,
                    "vt3": vt3,
                    "oT": oT,
                    "osb": None,
                }
            def emit_mm1(i):
                m, c, h = halves[i]
                ensure_loaded(m)
                st = slot_t[m]
                sc = psc.tile([128, 1024], f32, name="sc", tag="sc")
                sc_t[i] = sc
                kt_ap = st["kt"][:, 128 * c : 128 * (c + 1)]
                if m == 0 and c == 0:
                    kt_ap = fast0[:, 0:128]
                for jj in range(2):
                    if m == 0 and h == 0:
                        qt_ap = fast0[:, 128 + 512 * jj : 128 + 512 * (jj + 1)]
                    else:
                        qt_ap = st["qt"][
                            :, 1024 * h + 512 * jj : 1024 * h + 512 * (jj + 1)
                        ]
                    nc.tensor.matmul(
                        sc[:, 512 * jj : 512 * (jj + 1)],
                        kt_ap,
                        qt_ap,
                        start=True,
                        stop=True,
                    )

            def emit_exp(i):
                m, c, h = halves[i]
                st = slot_t[m]
                ex = expp.tile([128, 1024], bf16, name="ex", tag="ex")
                ex_t[i] = ex
                nc.scalar.activation(ex[:], sc_t[i][:], Exp, scale=0.125)
                sc_t[i] = None

            def emit_mm2(i):
                m, c, h = halves[i]
                s = sizes[m]
                st = slot_t[m]
                ex = ex_t[i]
                for jj in range(2):
                    nc.tensor.matmul(
                        st["oT"][2 * h + jj][:],
                        st["vt3"][:, c, :],
                        ex[:, 512 * jj : 512 * (jj + 1)],
                        start=(c == 0),
                        stop=(c == s - 1),
                    )
                ex_t[i] = None
                if c == s - 1:
                    # this half's oT pair is final: drain + DMA out this half
                    if st["osb"] is None:
                        st["osb"] = finp.tile([65, S], bf16, name="osb", tag="osb")
                    osb = st["osb"]
                    if m == K - 1 and h == 1:
                        # final half: split drain DVE + ScalarE (no exps left),
                        # quarter-outs on independent HWDGE queues
                        nc.vector.tensor_copy(
                            osb[:, 512 * 2 * h : 512 * (2 * h + 1)], st["oT"][2 * h][:]
                        )
                        nc.scalar.copy(
                            osb[:, 512 * (2 * h + 1) : 512 * (2 * h + 2)],
                            st["oT"][2 * h + 1][:],
                        )
                        nc.sync.dma_start(
                            out_d[m][:, 1024 * h : 1024 * (h + 1)],
                            osb[:, 1024 * h : 1024 * (h + 1)],
                        )
                    else:
                        for j in (2 * h, 2 * h + 1):
                            nc.vector.tensor_copy(
                                osb[:, 512 * j : 512 * (j + 1)], st["oT"][j][:]
                            )
                        nc.gpsimd.dma_start(
                            out_d[m][:, 1024 * h : 1024 * (h + 1)],
                            osb[:, 1024 * h : 1024 * (h + 1)],
                        )

            for i in range(-1, N + 2):
                j = i + 1
                if 0 <= j < N:
                    emit_mm1(j)
                if 0 <= i < N:
                    emit_exp(i)
                k2 = i - 2
                if 0 <= k2 < N:
                    emit_mm2(k2)

    nc.compile()
    return nc


def _get_nc(sizes=None):
    if sizes is None:
        sizes = _CACHE["sizes"]
    key = ("nc", sizes)
    if key not in _CACHE:
        _CACHE[key] = _build_nc(sizes)
    return _CACHE[key]


# --------------------------------------------------------------------- host


def make_in_maps(queries, keys, values, valid_lens):
    import ml_dtypes

    bf16 = ml_dtypes.bfloat16

    queries = np.asarray(queries, dtype=np.float32)
    keys = np.asarray(keys, dtype=np.float32)
    values = np.asarray(values, dtype=np.float32)
    valid_lens = np.asarray(valid_lens, dtype=np.int32)

    chunks = [int(-(-int(v) // 128)) for v in valid_lens]
    sizes, assign = _plan(chunks)
    _CACHE["sizes"] = sizes
    _CACHE["assign"] = assign

    # Per-batch precomputed panels, augmented with the mask row (row 64):
    # qT row 64 = 1.0; kT row 64 = 0 where key valid else NEG.
    qT = np.ones((B, 65, S), dtype=bf16)
    qT[:, 0:64] = queries.transpose(0, 2, 1).astype(bf16)
    kT = np.empty((B, 65, S), dtype=bf16)
    kT[:, 0:64] = keys.transpose(0, 2, 1).astype(bf16)
    kT[:, 64] = np.where(
        np.arange(S)[None, :] < valid_lens[:, None], 0.0, NEG
    ).astype(bf16)
    # V chunk-major with ones column: [B, 128, 16, 65]
    vt_full = np.ones((B, 128, NT, D + 1), dtype=bf16)
    vt_full[:, :, :, 0:D] = (
        values.reshape(B, NT, 128, D).transpose(0, 2, 1, 3).astype(bf16)
    )

    in_maps = []
    for core in range(N_CORES):
        im = {}
        for m, s in enumerate(sizes):
            piece = assign[core][m]
            qkp = np.zeros((65, s * 128 + S), dtype=bf16)
            qkp[64, 0 : s * 128] = bf16(NEG)  # padded keys stay masked
            vtbp = np.zeros((128, s * (D + 1)), dtype=bf16)
            if piece is not None:
                b, lo, ln = piece
                qkp[:, : ln * 128] = kT[b][:, lo * 128 : (lo + ln) * 128]
                qkp[:, s * 128 :] = qT[b]
                vtbp[:, : ln * (D + 1)] = vt_full[b, :, lo : lo + ln].reshape(128, -1)
            im[f"qk{m}"] = qkp
            im[f"vtb{m}"] = vtbp
            if m == 0:
                im["fast0"] = np.ascontiguousarray(
                    np.concatenate([qkp[:, 0:128], qkp[:, s * 128 : s * 128 + 1024]],
                                   axis=1)
                )
        in_maps.append(im)
    return in_maps


def run_on_device(in_maps, trace=False):
    from concourse.bass_utils import run_bass_kernel_spmd

    nc = _get_nc()
    return run_bass_kernel_spmd(
        nc, in_maps, core_ids=list(range(N_CORES)), trace=trace
    )


def combine(results):
    sizes = _CACHE["sizes"]
    assign = _CACHE["assign"]
    num = np.zeros((B, D, S), np.float32)
    den = np.zeros((B, S), np.float32)
    for core in range(N_CORES):
        r = results[core]
        for m in range(len(sizes)):
            piece = assign[core][m]
            if piece is None:
                continue
            b, lo, ln = piece
            part = np.asarray(r[f"out{m}"], dtype=np.float32)
            num[b] += part[0:64]
            den[b] += part[64]
    return np.ascontiguousarray((num / den[:, None, :]).transpose(0, 2, 1))


def kernel(**inputs):
    in_maps = make_in_maps(
        inputs["queries"], inputs["keys"], inputs["values"], inputs["valid_lens"]
    )
    res = run_on_device(in_maps, trace=False)
    return combine(res.results)


if __name__ == "__main__":
    chunks = [5, 7, 13, 1, 2, 7, 9, 16, 3, 2, 4, 1, 4, 3, 9, 8, 2, 7, 2, 7,
              16, 11, 7, 7, 4, 10, 15, 12, 2, 7, 4, 14]
    sizes, assign = _plan(chunks)
    print("sizes:", sizes)
    _build_nc(sizes)
    print("build OK")
